# revision 18
# baseline (speedup 1.0000x reference)
"""Trainium2 kernel for nn_PennyLaneQuantumClassifier.

Math: the quantum circuit is linear in the state vector, and the state is
amplitude-encoded from only N_INPUTS=10 real amplitudes.  Hence the PauliZ
expectation collapses to a quadratic form

    z0 = xs^T A xs / (xs^T xs),       xs = tanh(x * scale)

with A a 10x10 real symmetric matrix depending only on theta.  Using the
eigendecomposition A = V diag(lam) V^T (V orthogonal):

    g  = V^T xs
    t_j = sum((lam*w_j + b_j) * g^2)   (j = 0, 1)
    s   = sum(g^2)                      (= |xs|^2, V orthogonal)
    out_j = t_j / s

The device kernel streams x in a feature-on-partition packed layout
(8 row-chunks of 10 features stacked on 80 partitions): one ACT tanh,
one PE matvec (block-diag V), one ACT square, one PE reduction matmul
(t0/t1/s at 32-aligned partition groups), a reciprocal and two
interleaving multiplies.  Pure data-parallel across 8 NeuronCores.
"""

import numpy as np

N_QUBITS = 10
N_LAYERS = 4
N_INPUTS = 10
DIM = 2**N_QUBITS

BATCH = 32768
NCORES = 8
ROWS = BATCH // NCORES          # 4096 rows per core
C = 8                           # row-chunks stacked on partitions
NCOL = ROWS // C                # 512 columns (rows per chunk)
P = C * N_INPUTS                # 80 partitions used
NCONST = 1 + P + 96             # scale | bdv | red columns

T = 2                           # column tiles per core
RECIP_ENGINE = "act"            # "act" or "dve"

_PROG_CACHE: dict = {}


def _compute_A(theta: np.ndarray) -> np.ndarray:
    """Collapse the circuit: A[i,j] s.t. z0 = e^T A e for the embedded state."""
    th = theta.astype(np.float64).reshape(N_LAYERS, N_QUBITS, 3)
    a, b, c = th[..., 0], th[..., 1], th[..., 2]
    cb, sb = np.cos(b / 2), np.sin(b / 2)
    e = lambda t: np.exp(1j * t)
    u00 = e(-(a + c) / 2) * cb
    u01 = -1j * e((a - c) / 2) * sb
    u10 = -1j * e(-(a - c) / 2) * sb
    u11 = e((a + c) / 2) * cb
    U = np.stack([np.stack([u00, u01], -1), np.stack([u10, u11], -1)], -2)

    M = np.zeros((DIM, N_INPUTS), np.complex128)
    for i in range(N_INPUTS):
        M[i, i] = 1.0
    for l in range(N_LAYERS):
        for q in range(N_QUBITS):
            p = M.reshape(2**q, 2, -1, N_INPUTS)
            M = np.einsum("ab,qbri->qari", U[l, q], p).reshape(DIM, N_INPUTS)
        for q in range(N_QUBITS - 1):
            p = M.reshape(2**q, 2, 2, -1, N_INPUTS).copy()
            p[:, 1] = p[:, 1, ::-1]
            M = p.reshape(DIM, N_INPUTS)
    signs = np.concatenate([np.ones(DIM // 2), -np.ones(DIM // 2)])
    return np.real(M.conj().T @ (signs[:, None] * M))


def _act_reciprocal(nc, mybir, out, in_):
    """ACT Reciprocal without the bass accuracy guard (validated on HW)."""
    eng = nc.scalar
    return eng.add_instruction(
        mybir.InstActivation(
            name=nc.get_next_instruction_name(),
            func=mybir.ActivationFunctionType.Reciprocal,
            ins=[
                eng.lower_ap(in_),
                mybir.ImmediateValue(dtype=mybir.dt.float32, value=0.0),
                mybir.ImmediateValue(dtype=mybir.dt.float32, value=1.0),
                mybir.ImmediateValue(dtype=mybir.dt.float32, value=0.0),
            ],
            outs=[eng.lower_ap(out)],
        )
    )


def _build_program():
    import concourse.bacc as bacc
    import concourse.mybir as mybir
    from contextlib import ExitStack

    f32 = mybir.dt.float32
    f32r = mybir.dt.float32r
    W = NCOL // T
    Tanh = mybir.ActivationFunctionType.Tanh
    Square = mybir.ActivationFunctionType.Square

    nc = bacc.Bacc(trn_type="TRN2", target_bir_lowering=False, debug=False)
    x_d = nc.dram_tensor("xp", [P, NCOL], f32, kind="ExternalInput").ap()
    c_d = nc.dram_tensor("consts", [P, NCONST], f32r, kind="ExternalInput").ap()
    out_d = nc.dram_tensor("out", [ROWS, 2], f32, kind="ExternalOutput").ap()
    out_r = out_d.rearrange("(c j) o -> c (j o)", c=C)  # [C, 2*NCOL]

    warm = nc.alloc_sbuf_tensor("warm", [1, 1], f32).ap()
    xt = nc.alloc_sbuf_tensor("xt_raw", [P, NCOL], f32).ap()
    c_t = nc.alloc_sbuf_tensor("c_raw", [P, NCONST], f32r).ap()
    sc_ap = c_t[:, 0:1].bitcast(f32)
    v_ap = c_t[:, 1 : 1 + P]
    r_ap = c_t[:, 1 + P : 1 + P + 96]
    xs = [nc.alloc_sbuf_tensor(f"xs{t}", [P, W], f32r).ap() for t in range(T)]
    h = [nc.alloc_sbuf_tensor(f"h{t}", [P, W], f32r).ap() for t in range(T)]
    ss = [nc.alloc_sbuf_tensor(f"ss{t}", [C, W], f32).ap() for t in range(T)]
    rs = [nc.alloc_sbuf_tensor(f"rs{t}", [C, W], f32).ap() for t in range(T)]
    o = [nc.alloc_sbuf_tensor(f"o{t}", [C, 2 * W], f32).ap() for t in range(T)]

    in_x = nc.alloc_semaphore("in_x")
    in_c = nc.alloc_semaphore("in_c")
    out_sem = nc.alloc_semaphore("out_dma")
    act_sem = nc.alloc_semaphore("act")
    pe_sem = nc.alloc_semaphore("pe")
    dve_sem = nc.alloc_semaphore("dve")

    with ExitStack() as ctx:
        g = [
            ctx.enter_context(nc.psum_tensor(f"g{t}", [P, W], f32)).ap()
            for t in range(T)
        ]
        qs = [
            ctx.enter_context(nc.psum_tensor(f"qs{t}", [96, W], f32)).ap()
            for t in range(T)
        ]

        # SP: x half-tile DMA triggers (parallel HW queues), then gated
        # output DMAs
        for t in range(T):
            nc.sync.dma_start(
                xt[:, t * W : (t + 1) * W], x_d[:, t * W : (t + 1) * W]
            ).then_inc(in_x, 16)
        for t in range(T):
            nc.sync.dma_start(
                out_r[:, t * 2 * W : (t + 1) * 2 * W], o[t]
            )._wait_ge(dve_sem, 3 * (t + 1)).then_inc(out_sem, 16)
        nc.sync.wait_ge(out_sem, 32)

        # ACT: consts DMA on the second HWDGE engine, table warm-up, tanh,
        # square, s-copy.  act_sem counts from memzero.
        nc.scalar.dma_start(c_t, c_d).then_inc(in_c, 16)
        nc.scalar.memzero(warm).then_inc(act_sem, 1)
        nc.scalar.activation(warm, warm, Tanh).then_inc(act_sem, 1)
        nc.scalar.wait_ge(in_c, 16)
        nc.scalar.activation(
            xs[0], xt[:, 0:W], Tanh, scale=sc_ap
        )._wait_ge(in_x, 16).then_inc(act_sem, 1)  # act 3
        nc.scalar.activation(
            xs[1], xt[:, W : 2 * W], Tanh, scale=sc_ap
        )._wait_ge(in_x, 32).then_inc(act_sem, 1)  # act 4
        for t in range(T):
            nc.scalar.activation(h[t], g[t], Square)._wait_ge(
                pe_sem, t + 1
            ).then_inc(act_sem, 1)  # act 5, 6
        for t in range(T):
            nc.scalar.copy(ss[t], qs[t][64 : 64 + C, :])._wait_ge(
                pe_sem, 3 + t
            ).then_inc(act_sem, 1)  # act 7, 8

        # PE: two matvecs, two reductions
        for t in range(T):
            nc.tensor.matmul(
                g[t], v_ap, xs[t], start=True, stop=True
            )._wait_ge(act_sem, 3 + t).then_inc(pe_sem, 1)  # pe 1, 2
        for t in range(T):
            nc.tensor.matmul(
                qs[t], r_ap, h[t], start=True, stop=True
            )._wait_ge(act_sem, 5 + t).then_inc(pe_sem, 1)  # pe 3, 4

        # DVE: reciprocal + interleaving output muls
        for t in range(T):
            nc.vector.reciprocal_approx_fast(out=rs[t], in_=ss[t])._wait_ge(
                act_sem, 7 + t
            ).then_inc(dve_sem, 1)  # dve 1, 4
            nc.vector.tensor_mul(
                o[t][:, 0 : 2 * W : 2], qs[t][0:C, :], rs[t]
            ).then_inc(dve_sem, 1)  # dve 2, 5
            nc.vector.tensor_mul(
                o[t][:, 1 : 2 * W : 2], qs[t][32 : 32 + C, :], rs[t]
            ).then_inc(dve_sem, 1)  # dve 3, 6

        nc.compile()
    return nc


def _get_program():
    if "nc" not in _PROG_CACHE:
        _PROG_CACHE["nc"] = _build_program()
    return _PROG_CACHE["nc"]


def _host_constants(scale, theta, out_w, out_b):
    A = _compute_A(np.asarray(theta))
    lam, V = np.linalg.eigh(A)
    w = np.asarray(out_w, np.float64)[:, 0]
    b = np.asarray(out_b, np.float64)

    consts = np.zeros((P, NCONST), np.float64)
    consts[:, 0] = np.tile(np.asarray(scale, np.float64), C)
    consts[:, 1 : 1 + P] = np.kron(np.eye(C), V)
    red = np.zeros((P, 96), np.float64)
    for c in range(C):
        rows = slice(c * N_INPUTS, (c + 1) * N_INPUTS)
        red[rows, c] = lam * w[0] + b[0]
        red[rows, 32 + c] = lam * w[1] + b[1]
        red[rows, 64 + c] = 1.0
    consts[:, 1 + P : 1 + P + 96] = red
    return np.ascontiguousarray(consts.astype(np.float32))


def kernel(x, scale, theta, out_w, out_b, _trace=False):
    from concourse.bass_utils import run_bass_kernel_spmd

    x = np.ascontiguousarray(np.asarray(x, np.float32))
    consts = _host_constants(scale, theta, out_w, out_b)

    in_maps = []
    for k in range(NCORES):
        xc = x[k * ROWS : (k + 1) * ROWS]
        xp = np.ascontiguousarray(
            xc.reshape(C, NCOL, N_INPUTS).transpose(0, 2, 1).reshape(P, NCOL)
        )
        in_maps.append({"xp": xp, "consts": consts})

    nc = _get_program()
    res = run_bass_kernel_spmd(
        nc, in_maps, core_ids=list(range(NCORES)), trace=_trace
    )
    out = np.concatenate([res.results[k]["out"] for k in range(NCORES)], axis=0)
    if _trace:
        return out, res
    return out


# revision 31
# speedup vs baseline: 1.2762x; 1.2762x over previous
"""Trainium2 kernel for nn_PennyLaneQuantumClassifier.

Math: the quantum circuit is linear in the state vector, and the state is
amplitude-encoded from only N_INPUTS=10 real amplitudes.  Hence the PauliZ
expectation collapses to a quadratic form

    z0 = xs^T A xs / (xs^T xs),       xs = tanh(x * scale)

with A a 10x10 real symmetric matrix depending only on theta.  Using the
eigendecomposition A = V diag(lam) V^T (V orthogonal):

    g  = V^T xs
    t_j = sum((lam*w_j + b_j) * g^2)   (j = 0, 1)
    s   = sum(g^2)                      (= |xs|^2, V orthogonal)
    out_j = t_j / s

The device kernel streams x in a feature-on-partition packed layout
(8 row-chunks of 10 features stacked on 80 partitions): one ACT tanh,
one PE matvec (block-diag V), one ACT square, one PE reduction matmul
(t0/t1/s at 32-aligned partition groups), a reciprocal and two
interleaving multiplies.  Pure data-parallel across 8 NeuronCores.
"""

import numpy as np

N_QUBITS = 10
N_LAYERS = 4
N_INPUTS = 10
DIM = 2**N_QUBITS

BATCH = 32768
NCORES = 8
ROWS = BATCH // NCORES          # 4096 rows per core
C = 8                           # row-chunks stacked on partitions
NCOL = ROWS // C                # 512 columns (rows per chunk)
P = C * N_INPUTS                # 80 partitions used
NCONST = 1 + P + 96             # scale | bdv | red columns

T = 2                           # column tiles per core
END_WAIT = False                 # explicit wait for output DMA completion

_PROG_CACHE: dict = {}


def _compute_A(theta: np.ndarray) -> np.ndarray:
    """Collapse the circuit: A[i,j] s.t. z0 = e^T A e for the embedded state."""
    th = theta.astype(np.float64).reshape(N_LAYERS, N_QUBITS, 3)
    a, b, c = th[..., 0], th[..., 1], th[..., 2]
    cb, sb = np.cos(b / 2), np.sin(b / 2)
    e = lambda t: np.exp(1j * t)
    u00 = e(-(a + c) / 2) * cb
    u01 = -1j * e((a - c) / 2) * sb
    u10 = -1j * e(-(a - c) / 2) * sb
    u11 = e((a + c) / 2) * cb
    U = np.stack([np.stack([u00, u01], -1), np.stack([u10, u11], -1)], -2)

    M = np.zeros((DIM, N_INPUTS), np.complex128)
    for i in range(N_INPUTS):
        M[i, i] = 1.0
    for l in range(N_LAYERS):
        for q in range(N_QUBITS):
            p = M.reshape(2**q, 2, -1, N_INPUTS)
            M = np.einsum("ab,qbri->qari", U[l, q], p).reshape(DIM, N_INPUTS)
        for q in range(N_QUBITS - 1):
            p = M.reshape(2**q, 2, 2, -1, N_INPUTS).copy()
            p[:, 1] = p[:, 1, ::-1]
            M = p.reshape(DIM, N_INPUTS)
    signs = np.concatenate([np.ones(DIM // 2), -np.ones(DIM // 2)])
    return np.real(M.conj().T @ (signs[:, None] * M))


def _act_reciprocal(nc, mybir, out, in_):
    """ACT Reciprocal without the bass accuracy guard (validated on HW)."""
    eng = nc.scalar
    return eng.add_instruction(
        mybir.InstActivation(
            name=nc.get_next_instruction_name(),
            func=mybir.ActivationFunctionType.Reciprocal,
            ins=[
                eng.lower_ap(in_),
                mybir.ImmediateValue(dtype=mybir.dt.float32, value=0.0),
                mybir.ImmediateValue(dtype=mybir.dt.float32, value=1.0),
                mybir.ImmediateValue(dtype=mybir.dt.float32, value=0.0),
            ],
            outs=[eng.lower_ap(out)],
        )
    )


def _build_program():
    import concourse.bacc as bacc
    import concourse.mybir as mybir
    from contextlib import ExitStack

    f32 = mybir.dt.float32
    f32r = mybir.dt.float32r
    W = NCOL // T
    Tanh = mybir.ActivationFunctionType.Tanh
    Square = mybir.ActivationFunctionType.Square

    nc = bacc.Bacc(trn_type="TRN2", target_bir_lowering=False, debug=False)
    x_d = nc.dram_tensor("xp", [P, NCOL], f32, kind="ExternalInput").ap()
    sc_d = nc.dram_tensor("scale_p", [P, 1], f32, kind="ExternalInput").ap()
    vr_d = nc.dram_tensor("vr", [P, P + P], f32r, kind="ExternalInput").ap()
    op_d = nc.dram_tensor("outp", [2 * C, NCOL], f32, kind="ExternalOutput").ap()

    warm = nc.alloc_sbuf_tensor("warm", [1, 1], f32).ap()
    xt = nc.alloc_sbuf_tensor("xt_raw", [P, NCOL], f32).ap()
    sc_t = nc.alloc_sbuf_tensor("sc_raw", [P, 1], f32).ap()
    vr_t = nc.alloc_sbuf_tensor("vr_raw", [P, P + P], f32r).ap()
    sc_ap = sc_t
    v_ap = vr_t[:, 0:P]
    r_ap = vr_t[:, P : P + P]
    xs = [nc.alloc_sbuf_tensor(f"xs{t}", [P, W], f32r).ap() for t in range(T)]
    h = [nc.alloc_sbuf_tensor(f"h{t}", [P, W], f32r).ap() for t in range(T)]
    ss = [nc.alloc_sbuf_tensor(f"ss{t}", [2 * C, W], f32).ap() for t in range(T)]
    rs = [nc.alloc_sbuf_tensor(f"rs{t}", [2 * C, W], f32).ap() for t in range(T)]
    o = [nc.alloc_sbuf_tensor(f"o{t}", [2 * C, W], f32).ap() for t in range(T)]

    in_x = nc.alloc_semaphore("in_x")
    in_sc = nc.alloc_semaphore("in_sc")
    in_vr = nc.alloc_semaphore("in_vr")
    out_sem = nc.alloc_semaphore("out_dma")
    act_sem = nc.alloc_semaphore("act")
    pe_sem = nc.alloc_semaphore("pe")
    dve_sem = nc.alloc_semaphore("dve")

    with ExitStack() as ctx:
        g = [
            ctx.enter_context(nc.psum_tensor(f"g{t}", [P, W], f32)).ap()
            for t in range(T)
        ]
        qs = [
            ctx.enter_context(nc.psum_tensor(f"qs{t}", [P, W], f32)).ap()
            for t in range(T)
        ]

        # SP: x half-tile DMA triggers (parallel HW queues), then gated
        # output DMAs (compact per-component halves; host interleaves)
        for t in range(T):
            nc.sync.dma_start(
                xt[:, t * W : (t + 1) * W], x_d[:, t * W : (t + 1) * W]
            ).then_inc(in_x, 16)
        for t in range(T):
            sl = slice(t * W, (t + 1) * W)
            nc.sync.dma_start(op_d[:, sl], o[t])._wait_ge(
                dve_sem, 2 * (t + 1)
            ).then_inc(out_sem, 16)
        if END_WAIT:
            nc.sync.wait_ge(out_sem, 32)

        # ACT: scale + weights DMAs on the second HWDGE engine, table
        # warm-up, tanh, square, s-copy.  act_sem counts from memzero.
        nc.scalar.dma_start(sc_t, sc_d).then_inc(in_sc, 16)
        nc.scalar.dma_start(vr_t, vr_d).then_inc(in_vr, 16)
        nc.scalar.memzero(warm).then_inc(act_sem, 1)
        nc.scalar.activation(warm, warm, Tanh).then_inc(act_sem, 1)
        nc.scalar.wait_ge(in_sc, 16)
        nc.scalar.activation(
            xs[0], xt[:, 0:W], Tanh, scale=sc_ap
        )._wait_ge(in_x, 16).then_inc(act_sem, 1)  # act 3
        nc.scalar.activation(
            xs[1], xt[:, W : 2 * W], Tanh, scale=sc_ap
        )._wait_ge(in_x, 32).then_inc(act_sem, 1)  # act 4
        for t in range(T):
            nc.scalar.activation(h[t], g[t], Square)._wait_ge(
                pe_sem, t + 1
            ).then_inc(act_sem, 1)  # act 5, 6
        for t in range(T):
            nc.scalar.copy(ss[t], qs[t][64 : 64 + 2 * C, :])._wait_ge(
                pe_sem, 3 + t
            ).then_inc(act_sem, 1)  # act 7, 8

        # PE: two matvecs, two reductions
        nc.tensor.wait_ge(in_vr, 16)
        for t in range(T):
            nc.tensor.matmul(
                g[t], v_ap, xs[t], start=True, stop=True
            )._wait_ge(act_sem, 3 + t).then_inc(pe_sem, 1)  # pe 1, 2
        for t in range(T):
            nc.tensor.matmul(
                qs[t], r_ap, h[t], start=True, stop=True
            )._wait_ge(act_sem, 5 + t).then_inc(pe_sem, 1)  # pe 3, 4

        # DVE: reciprocal on the duplicated s rows + one paired output mul
        for t in range(T):
            nc.vector.reciprocal_approx_fast(out=rs[t], in_=ss[t])._wait_ge(
                act_sem, 7 + t
            ).then_inc(dve_sem, 1)  # dve 1, 3
            nc.vector.tensor_mul(
                o[t], qs[t][0 : 2 * C, :], rs[t]
            ).then_inc(dve_sem, 1)  # dve 2, 4

        nc.compile()
    return nc


def _get_program():
    if "nc" not in _PROG_CACHE:
        _PROG_CACHE["nc"] = _build_program()
    return _PROG_CACHE["nc"]


def _host_constants(scale, theta, out_w, out_b):
    A = _compute_A(np.asarray(theta))
    lam, V = np.linalg.eigh(A)
    w = np.asarray(out_w, np.float64)[:, 0]
    b = np.asarray(out_b, np.float64)

    scale_p = np.tile(np.asarray(scale, np.float64), C)[:, None]
    vr = np.zeros((P, P + P), np.float64)
    vr[:, 0:P] = np.kron(np.eye(C), V)
    for c in range(C):
        rows = slice(c * N_INPUTS, (c + 1) * N_INPUTS)
        vr[rows, P + c] = lam * w[0] + b[0]
        vr[rows, P + C + c] = lam * w[1] + b[1]
        vr[rows, P + 64 + c] = 1.0
        vr[rows, P + 64 + C + c] = 1.0
    return (np.ascontiguousarray(scale_p.astype(np.float32)),
            np.ascontiguousarray(vr.astype(np.float32)))


def kernel(x, scale, theta, out_w, out_b, _trace=False):
    from concourse.bass_utils import run_bass_kernel_spmd

    x = np.ascontiguousarray(np.asarray(x, np.float32))
    scale_p, vr = _host_constants(scale, theta, out_w, out_b)

    in_maps = []
    for k in range(NCORES):
        xc = x[k * ROWS : (k + 1) * ROWS]
        xp = np.ascontiguousarray(
            xc.reshape(C, NCOL, N_INPUTS).transpose(0, 2, 1).reshape(P, NCOL)
        )
        in_maps.append({"xp": xp, "scale_p": scale_p, "vr": vr})

    nc = _get_program()
    res = run_bass_kernel_spmd(
        nc, in_maps, core_ids=list(range(NCORES)), trace=_trace
    )
    parts = []
    for k in range(NCORES):
        op = res.results[k]["outp"]
        parts.append(np.stack([op[0:C].reshape(ROWS), op[C:].reshape(ROWS)], -1))
    out = np.concatenate(parts, axis=0)
    if _trace:
        return out, res
    return out


# revision 36
# speedup vs baseline: 1.3246x; 1.0379x over previous
"""Trainium2 kernel for nn_PennyLaneQuantumClassifier.

Math: the quantum circuit is linear in the state vector, and the state is
amplitude-encoded from only N_INPUTS=10 real amplitudes.  Hence the PauliZ
expectation collapses to a quadratic form

    z0 = xs^T A xs / (xs^T xs),       xs = tanh(x * scale)

with A a 10x10 real symmetric matrix depending only on theta.  Using the
eigendecomposition A = V diag(lam) V^T (V orthogonal):

    g  = V^T xs
    t_j = sum((lam*w_j + b_j) * g^2)   (j = 0, 1)
    s   = sum(g^2)                      (= |xs|^2, V orthogonal)
    out_j = t_j / s

The device kernel (raw bacc, manual semaphores) streams x in a
feature-on-partition packed layout (8 row-chunks of 10 features stacked on
80 partitions, scale prepended as column 0 of the x DMA).  Per column tile:
ACT tanh -> PE matvec (block-diag V, f32r) -> ACT square -> PE reduction
matmul (t0 rows 0-7, t1 rows 8-15, s duplicated at rows 64-79) -> ACT copy
of s to SBUF -> DVE 1-pass reciprocal -> one paired DVE multiply -> one
output DMA per tile (component-major; host interleaves the two output
columns during the gather).  Pure data-parallel across 8 NeuronCores.
"""

import numpy as np

N_QUBITS = 10
N_LAYERS = 4
N_INPUTS = 10
DIM = 2**N_QUBITS

BATCH = 32768
NCORES = 8
ROWS = BATCH // NCORES          # 4096 rows per core
C = 8                           # row-chunks stacked on partitions
NCOL = ROWS // C                # 512 columns (rows per chunk)
P = C * N_INPUTS                # 80 partitions used
NCONST = 1 + P + 96             # scale | bdv | red columns

T = 2                           # column tiles per core
END_WAIT = False                 # explicit wait for output DMA completion

_PROG_CACHE: dict = {}


def _compute_A(theta: np.ndarray) -> np.ndarray:
    """Collapse the circuit: A[i,j] s.t. z0 = e^T A e for the embedded state."""
    th = theta.astype(np.float64).reshape(N_LAYERS, N_QUBITS, 3)
    a, b, c = th[..., 0], th[..., 1], th[..., 2]
    cb, sb = np.cos(b / 2), np.sin(b / 2)
    e = lambda t: np.exp(1j * t)
    u00 = e(-(a + c) / 2) * cb
    u01 = -1j * e((a - c) / 2) * sb
    u10 = -1j * e(-(a - c) / 2) * sb
    u11 = e((a + c) / 2) * cb
    U = np.stack([np.stack([u00, u01], -1), np.stack([u10, u11], -1)], -2)

    M = np.zeros((DIM, N_INPUTS), np.complex128)
    for i in range(N_INPUTS):
        M[i, i] = 1.0
    for l in range(N_LAYERS):
        for q in range(N_QUBITS):
            p = M.reshape(2**q, 2, -1, N_INPUTS)
            M = np.einsum("ab,qbri->qari", U[l, q], p).reshape(DIM, N_INPUTS)
        for q in range(N_QUBITS - 1):
            p = M.reshape(2**q, 2, 2, -1, N_INPUTS).copy()
            p[:, 1] = p[:, 1, ::-1]
            M = p.reshape(DIM, N_INPUTS)
    signs = np.concatenate([np.ones(DIM // 2), -np.ones(DIM // 2)])
    return np.real(M.conj().T @ (signs[:, None] * M))


def _act_reciprocal(nc, mybir, out, in_):
    """ACT Reciprocal without the bass accuracy guard (validated on HW)."""
    eng = nc.scalar
    return eng.add_instruction(
        mybir.InstActivation(
            name=nc.get_next_instruction_name(),
            func=mybir.ActivationFunctionType.Reciprocal,
            ins=[
                eng.lower_ap(in_),
                mybir.ImmediateValue(dtype=mybir.dt.float32, value=0.0),
                mybir.ImmediateValue(dtype=mybir.dt.float32, value=1.0),
                mybir.ImmediateValue(dtype=mybir.dt.float32, value=0.0),
            ],
            outs=[eng.lower_ap(out)],
        )
    )


def _build_program():
    import concourse.bacc as bacc
    import concourse.mybir as mybir
    from contextlib import ExitStack

    f32 = mybir.dt.float32
    f32r = mybir.dt.float32r
    WS = [256, 256]
    OFF = [0, 256]
    Tanh = mybir.ActivationFunctionType.Tanh
    Square = mybir.ActivationFunctionType.Square

    nc = bacc.Bacc(trn_type="TRN2", target_bir_lowering=False, debug=False)
    x_d = nc.dram_tensor("xp", [P, NCOL + 1], f32, kind="ExternalInput").ap()
    vr_d = nc.dram_tensor("vr", [P, P + P], f32r, kind="ExternalInput").ap()
    op_d = nc.dram_tensor("outp", [2 * C, NCOL], f32, kind="ExternalOutput").ap()

    warm = nc.alloc_sbuf_tensor("warm", [1, 1], f32).ap()
    xt = nc.alloc_sbuf_tensor("xt_raw", [P, NCOL + 1], f32).ap()
    vr_t = nc.alloc_sbuf_tensor("vr_raw", [P, P + P], f32r).ap()
    sc_ap = xt[:, 0:1]
    v_ap = vr_t[:, 0:P]
    r_ap = vr_t[:, P : P + P]
    xs = [nc.alloc_sbuf_tensor(f"xs{t}", [P, WS[t]], f32r).ap() for t in range(T)]
    h = [nc.alloc_sbuf_tensor(f"h{t}", [P, WS[t]], f32r).ap() for t in range(T)]
    ss = [nc.alloc_sbuf_tensor(f"ss{t}", [2 * C, WS[t]], f32).ap() for t in range(T)]
    rs = [nc.alloc_sbuf_tensor(f"rs{t}", [2 * C, WS[t]], f32).ap() for t in range(T)]
    o = [nc.alloc_sbuf_tensor(f"o{t}", [2 * C, WS[t]], f32).ap() for t in range(T)]

    in_x = nc.alloc_semaphore("in_x")
    in_sc = nc.alloc_semaphore("in_sc")
    in_vr = nc.alloc_semaphore("in_vr")
    out_sem = nc.alloc_semaphore("out_dma")
    act_sem = nc.alloc_semaphore("act")
    pe_sem = nc.alloc_semaphore("pe")
    dve_sem = nc.alloc_semaphore("dve")

    with ExitStack() as ctx:
        g = [
            ctx.enter_context(nc.psum_tensor(f"g{t}", [P, WS[t]], f32)).ap()
            for t in range(T)
        ]
        qs = [
            ctx.enter_context(nc.psum_tensor(f"qs{t}", [P, WS[t]], f32)).ap()
            for t in range(T)
        ]

        # SP: x half-tile DMA triggers (parallel HW queues), then gated
        # output DMAs (compact per-component halves; host interleaves)
        nc.sync.dma_start(
            xt[:, 0 : WS[0] + 1], x_d[:, 0 : WS[0] + 1]
        ).then_inc(in_x, 16)
        nc.sync.dma_start(
            xt[:, WS[0] + 1 : NCOL + 1], x_d[:, WS[0] + 1 : NCOL + 1]
        ).then_inc(in_x, 16)
        for t in range(T):
            sl = slice(OFF[t], OFF[t] + WS[t])
            nc.sync.dma_start(op_d[:, sl], o[t])._wait_ge(
                dve_sem, 2 * (t + 1)
            ).then_inc(out_sem, 16)
        if END_WAIT:
            nc.sync.wait_ge(out_sem, 32)

        # ACT: scale + weights DMAs on the second HWDGE engine, table
        # warm-up, tanh, square, s-copy.  act_sem counts from memzero.
        nc.scalar.dma_start(vr_t, vr_d).then_inc(in_vr, 16)
        nc.scalar.memzero(warm).then_inc(act_sem, 1)
        nc.scalar.activation(warm, warm, Tanh).then_inc(act_sem, 1)
        nc.scalar.activation(
            xs[0], xt[:, 1 : WS[0] + 1], Tanh, scale=sc_ap
        )._wait_ge(in_x, 16).then_inc(act_sem, 1)  # act 3
        nc.scalar.activation(
            xs[1], xt[:, WS[0] + 1 : NCOL + 1], Tanh, scale=sc_ap
        )._wait_ge(in_x, 32).then_inc(act_sem, 1)  # act 4
        for t in range(T):
            nc.scalar.activation(h[t], g[t], Square)._wait_ge(
                pe_sem, t + 1
            ).then_inc(act_sem, 1)  # act 5, 6
        for t in range(T):
            nc.scalar.copy(ss[t], qs[t][64 : 64 + 2 * C, :])._wait_ge(
                pe_sem, 3 + t
            ).then_inc(act_sem, 1)  # act 7, 8

        # PE: two matvecs, two reductions
        nc.tensor.wait_ge(in_vr, 16)
        for t in range(T):
            nc.tensor.matmul(
                g[t], v_ap, xs[t], start=True, stop=True
            )._wait_ge(act_sem, 3 + t).then_inc(pe_sem, 1)  # pe 1, 2
        for t in range(T):
            nc.tensor.matmul(
                qs[t], r_ap, h[t], start=True, stop=True
            )._wait_ge(act_sem, 5 + t).then_inc(pe_sem, 1)  # pe 3, 4

        # DVE: reciprocal on the duplicated s rows + one paired output mul
        for t in range(T):
            nc.vector.reciprocal_approx_fast(out=rs[t], in_=ss[t])._wait_ge(
                act_sem, 7 + t
            ).then_inc(dve_sem, 1)  # dve 1, 3
            nc.vector.tensor_mul(
                o[t], qs[t][0 : 2 * C, :], rs[t]
            ).then_inc(dve_sem, 1)  # dve 2, 4

        nc.compile()
    return nc


def _get_program():
    if "nc" not in _PROG_CACHE:
        _PROG_CACHE["nc"] = _build_program()
    return _PROG_CACHE["nc"]


def _host_constants(scale, theta, out_w, out_b):
    A = _compute_A(np.asarray(theta))
    lam, V = np.linalg.eigh(A)
    w = np.asarray(out_w, np.float64)[:, 0]
    b = np.asarray(out_b, np.float64)

    scale_p = np.tile(np.asarray(scale, np.float64), C)[:, None]
    vr = np.zeros((P, P + P), np.float64)
    vr[:, 0:P] = np.kron(np.eye(C), V)
    for c in range(C):
        rows = slice(c * N_INPUTS, (c + 1) * N_INPUTS)
        vr[rows, P + c] = lam * w[0] + b[0]
        vr[rows, P + C + c] = lam * w[1] + b[1]
        vr[rows, P + 64 + c] = 1.0
        vr[rows, P + 64 + C + c] = 1.0
    return (np.ascontiguousarray(scale_p.astype(np.float32)),
            np.ascontiguousarray(vr.astype(np.float32)))


def kernel(x, scale, theta, out_w, out_b, _trace=False):
    from concourse.bass_utils import run_bass_kernel_spmd

    x = np.ascontiguousarray(np.asarray(x, np.float32))
    scale_p, vr = _host_constants(scale, theta, out_w, out_b)

    in_maps = []
    for k in range(NCORES):
        xc = x[k * ROWS : (k + 1) * ROWS]
        xp = xc.reshape(C, NCOL, N_INPUTS).transpose(0, 2, 1).reshape(P, NCOL)
        xp = np.ascontiguousarray(np.concatenate([scale_p, xp], axis=1))
        in_maps.append({"xp": xp, "vr": vr})

    nc = _get_program()
    res = run_bass_kernel_spmd(
        nc, in_maps, core_ids=list(range(NCORES)), trace=_trace
    )
    parts = []
    for k in range(NCORES):
        op = res.results[k]["outp"]
        parts.append(np.stack([op[0:C].reshape(ROWS), op[C:].reshape(ROWS)], -1))
    out = np.concatenate(parts, axis=0)
    if _trace:
        return out, res
    return out


# revision 38
# speedup vs baseline: 1.3268x; 1.0017x over previous
"""Trainium2 kernel for nn_PennyLaneQuantumClassifier.

Math: the quantum circuit is linear in the state vector, and the state is
amplitude-encoded from only N_INPUTS=10 real amplitudes.  Hence the PauliZ
expectation collapses to a quadratic form

    z0 = xs^T A xs / (xs^T xs),       xs = tanh(x * scale)

with A a 10x10 real symmetric matrix depending only on theta.  Using the
eigendecomposition A = V diag(lam) V^T (V orthogonal):

    g  = V^T xs
    t_j = sum((lam*w_j + b_j) * g^2)   (j = 0, 1)
    s   = sum(g^2)                      (= |xs|^2, V orthogonal)
    out_j = t_j / s

The device kernel (raw bacc, manual semaphores) streams x in a
feature-on-partition packed layout (8 row-chunks of 10 features stacked on
80 partitions, scale prepended as column 0 of the x DMA).  Per column tile:
ACT tanh -> PE matvec (block-diag V, f32r) -> ACT square -> PE reduction
matmul (t0 rows 0-7, t1 rows 8-15, s duplicated at rows 64-79) -> ACT copy
of s to SBUF -> DVE 1-pass reciprocal -> one paired DVE multiply -> one
output DMA per tile (component-major; host interleaves the two output
columns during the gather).  Pure data-parallel across 8 NeuronCores.
"""

import numpy as np

N_QUBITS = 10
N_LAYERS = 4
N_INPUTS = 10
DIM = 2**N_QUBITS

BATCH = 32768
NCORES = 8
ROWS = BATCH // NCORES          # 4096 rows per core
C = 8                           # row-chunks stacked on partitions
NCOL = ROWS // C                # 512 columns (rows per chunk)
P = C * N_INPUTS                # 80 partitions used
NCONST = 1 + P + 96             # scale | bdv | red columns

T = 2                           # column tiles per core
END_WAIT = False                 # explicit wait for output DMA completion

_PROG_CACHE: dict = {}


def _compute_A(theta: np.ndarray) -> np.ndarray:
    """Collapse the circuit: A[i,j] s.t. z0 = e^T A e for the embedded state."""
    th = theta.astype(np.float64).reshape(N_LAYERS, N_QUBITS, 3)
    a, b, c = th[..., 0], th[..., 1], th[..., 2]
    cb, sb = np.cos(b / 2), np.sin(b / 2)
    e = lambda t: np.exp(1j * t)
    u00 = e(-(a + c) / 2) * cb
    u01 = -1j * e((a - c) / 2) * sb
    u10 = -1j * e(-(a - c) / 2) * sb
    u11 = e((a + c) / 2) * cb
    U = np.stack([np.stack([u00, u01], -1), np.stack([u10, u11], -1)], -2)

    M = np.zeros((DIM, N_INPUTS), np.complex128)
    for i in range(N_INPUTS):
        M[i, i] = 1.0
    for l in range(N_LAYERS):
        for q in range(N_QUBITS):
            p = M.reshape(2**q, 2, -1, N_INPUTS)
            M = np.einsum("ab,qbri->qari", U[l, q], p).reshape(DIM, N_INPUTS)
        for q in range(N_QUBITS - 1):
            p = M.reshape(2**q, 2, 2, -1, N_INPUTS).copy()
            p[:, 1] = p[:, 1, ::-1]
            M = p.reshape(DIM, N_INPUTS)
    signs = np.concatenate([np.ones(DIM // 2), -np.ones(DIM // 2)])
    return np.real(M.conj().T @ (signs[:, None] * M))


def _act_reciprocal(nc, mybir, out, in_):
    """ACT Reciprocal without the bass accuracy guard (validated on HW)."""
    eng = nc.scalar
    return eng.add_instruction(
        mybir.InstActivation(
            name=nc.get_next_instruction_name(),
            func=mybir.ActivationFunctionType.Reciprocal,
            ins=[
                eng.lower_ap(in_),
                mybir.ImmediateValue(dtype=mybir.dt.float32, value=0.0),
                mybir.ImmediateValue(dtype=mybir.dt.float32, value=1.0),
                mybir.ImmediateValue(dtype=mybir.dt.float32, value=0.0),
            ],
            outs=[eng.lower_ap(out)],
        )
    )


def _build_program():
    import concourse.bacc as bacc
    import concourse.mybir as mybir
    from contextlib import ExitStack

    f32 = mybir.dt.float32
    f32r = mybir.dt.float32r
    WS = [256, 256]
    OFF = [0, 256]
    Tanh = mybir.ActivationFunctionType.Tanh
    Square = mybir.ActivationFunctionType.Square

    nc = bacc.Bacc(trn_type="TRN2", target_bir_lowering=False, debug=False)
    x_d = nc.dram_tensor("xp", [P, NCOL + 1], f32, kind="ExternalInput").ap()
    vr_d = nc.dram_tensor("vr", [P, P + P], f32r, kind="ExternalInput").ap()
    op_d = nc.dram_tensor("outp", [2 * C, NCOL], f32, kind="ExternalOutput").ap()

    warm = nc.alloc_sbuf_tensor("warm", [1, 1], f32).ap()
    xt = nc.alloc_sbuf_tensor("xt_raw", [P, NCOL + 1], f32).ap()
    vr_t = nc.alloc_sbuf_tensor("vr_raw", [P, P + P], f32r).ap()
    sc_ap = xt[:, 0:1]
    v_ap = vr_t[:, 0:P]
    r_ap = vr_t[:, P : P + P]
    xs = [nc.alloc_sbuf_tensor(f"xs{t}", [P, WS[t]], f32r).ap() for t in range(T)]
    h = [nc.alloc_sbuf_tensor(f"h{t}", [P, WS[t]], f32r).ap() for t in range(T)]
    ss = [nc.alloc_sbuf_tensor(f"ss{t}", [2 * C, WS[t]], f32).ap() for t in range(T)]
    rs = [nc.alloc_sbuf_tensor(f"rs{t}", [2 * C, WS[t]], f32).ap() for t in range(T)]
    o = [nc.alloc_sbuf_tensor(f"o{t}", [2 * C, WS[t]], f32).ap() for t in range(T)]

    in_x = nc.alloc_semaphore("in_x")
    in_sc = nc.alloc_semaphore("in_sc")
    in_vr = nc.alloc_semaphore("in_vr")
    out_sem = nc.alloc_semaphore("out_dma")
    act_sem = nc.alloc_semaphore("act")
    pe_sem = nc.alloc_semaphore("pe")
    dve_sem = nc.alloc_semaphore("dve")

    with ExitStack() as ctx:
        g = [
            ctx.enter_context(nc.psum_tensor(f"g{t}", [P, WS[t]], f32)).ap()
            for t in range(T)
        ]
        qs = [
            ctx.enter_context(nc.psum_tensor(f"qs{t}", [P, WS[t]], f32)).ap()
            for t in range(T)
        ]

        # SP: x half-tile DMA triggers (parallel HW queues), then gated
        # output DMAs (compact per-component halves; host interleaves)
        nc.sync.dma_start(
            xt[:, 0 : WS[0] + 1], x_d[:, 0 : WS[0] + 1]
        ).then_inc(in_x, 16)
        nc.sync.dma_start(
            xt[:, WS[0] + 1 : NCOL + 1], x_d[:, WS[0] + 1 : NCOL + 1]
        ).then_inc(in_x, 16)
        for t in range(T):
            nc.sync.dma_start(
                op_d[:, OFF[t] : OFF[t] + WS[t]], o[t]
            )._wait_ge(dve_sem, 2 * (t + 1)).then_inc(out_sem, 16)
        if END_WAIT:
            nc.sync.wait_ge(out_sem, 32)

        # ACT: scale + weights DMAs on the second HWDGE engine, table
        # warm-up, tanh, square, s-copy.  act_sem counts from memzero.
        nc.scalar.dma_start(vr_t, vr_d).then_inc(in_vr, 16)
        nc.scalar.memzero(warm).then_inc(act_sem, 1)
        nc.scalar.activation(warm, warm, Tanh).then_inc(act_sem, 1)
        nc.scalar.activation(
            xs[0], xt[:, 1 : WS[0] + 1], Tanh, scale=sc_ap
        )._wait_ge(in_x, 16).then_inc(act_sem, 1)  # act 3
        nc.scalar.activation(
            xs[1], xt[:, WS[0] + 1 : NCOL + 1], Tanh, scale=sc_ap
        )._wait_ge(in_x, 32).then_inc(act_sem, 1)  # act 4
        for t in range(T):
            nc.scalar.activation(h[t], g[t], Square)._wait_ge(
                pe_sem, t + 1
            ).then_inc(act_sem, 1)  # act 5, 6
        for t in range(T):
            nc.scalar.copy(ss[t], qs[t][64 : 64 + 2 * C, :])._wait_ge(
                pe_sem, 3 + t
            ).then_inc(act_sem, 1)  # act 7, 8

        # PE: two matvecs, two reductions
        nc.tensor.wait_ge(in_vr, 16)
        for t in range(T):
            nc.tensor.matmul(
                g[t], v_ap, xs[t], start=True, stop=True
            )._wait_ge(act_sem, 3 + t).then_inc(pe_sem, 1)  # pe 1, 2
        for t in range(T):
            nc.tensor.matmul(
                qs[t], r_ap, h[t], start=True, stop=True
            )._wait_ge(act_sem, 5 + t).then_inc(pe_sem, 1)  # pe 3, 4

        # DVE: reciprocal on the duplicated s rows + one paired output mul
        for t in range(T):
            nc.vector.reciprocal_approx_fast(out=rs[t], in_=ss[t])._wait_ge(
                act_sem, 7 + t
            ).then_inc(dve_sem, 1)  # dve 1, 3
            nc.vector.tensor_mul(
                o[t], qs[t][0 : 2 * C, :], rs[t]
            ).then_inc(dve_sem, 1)  # dve 2, 4

        nc.compile()
    return nc


def _get_program():
    if "nc" not in _PROG_CACHE:
        _PROG_CACHE["nc"] = _build_program()
    return _PROG_CACHE["nc"]


def _host_constants(scale, theta, out_w, out_b):
    A = _compute_A(np.asarray(theta))
    lam, V = np.linalg.eigh(A)
    w = np.asarray(out_w, np.float64)[:, 0]
    b = np.asarray(out_b, np.float64)

    scale_p = np.tile(np.asarray(scale, np.float64), C)[:, None]
    vr = np.zeros((P, P + P), np.float64)
    vr[:, 0:P] = np.kron(np.eye(C), V)
    for c in range(C):
        rows = slice(c * N_INPUTS, (c + 1) * N_INPUTS)
        vr[rows, P + c] = lam * w[0] + b[0]
        vr[rows, P + C + c] = lam * w[1] + b[1]
        vr[rows, P + 64 + c] = 1.0
        vr[rows, P + 64 + C + c] = 1.0
    return (np.ascontiguousarray(scale_p.astype(np.float32)),
            np.ascontiguousarray(vr.astype(np.float32)))


def kernel(x, scale, theta, out_w, out_b, _trace=False):
    from concourse.bass_utils import run_bass_kernel_spmd

    x = np.ascontiguousarray(np.asarray(x, np.float32))
    scale_p, vr = _host_constants(scale, theta, out_w, out_b)

    in_maps = []
    for k in range(NCORES):
        xc = x[k * ROWS : (k + 1) * ROWS]
        xp = xc.reshape(C, NCOL, N_INPUTS).transpose(0, 2, 1).reshape(P, NCOL)
        xp = np.ascontiguousarray(np.concatenate([scale_p, xp], axis=1))
        in_maps.append({"xp": xp, "vr": vr})

    nc = _get_program()
    res = run_bass_kernel_spmd(
        nc, in_maps, core_ids=list(range(NCORES)), trace=_trace
    )
    parts = []
    for k in range(NCORES):
        op = res.results[k]["outp"]
        parts.append(np.stack([op[0:C].reshape(ROWS), op[C:].reshape(ROWS)], -1))
    out = np.concatenate(parts, axis=0)
    if _trace:
        return out, res
    return out
